# revision 2
# baseline (speedup 1.0000x reference)
"""Multi-head self-attention (causal) Trainium2 Bass/Tile kernel, 8-way SPMD.

Sharding: data-parallel over batch (4) x tensor-parallel over heads (2 groups
of 8 heads).  Core c handles batch c//2, head-group c%2.  Each core computes
q/k/v projections for its 512 local features, causal attention for its 8
heads, and a partial o-projection (contraction over its 512 features of the
attention output) giving a full-shape [S, D] partial that the host sums per
batch pair.

All matmul operands are bf16 (fp32 PSUM accumulation); softmax runs without
max-subtraction (scores ~ N(0,1) after the 1/8 scale, no overflow risk), with
exp on the scalar engine and the row-sum folded into the AV matmul via a ones
column appended to V.  Host pre-transposes inputs so no on-chip transposes
are needed:
  qT[e,s]  = wqT.T @ xT        (lhsT=wqT[d,e], rhs=xT[d,s])
  scoresT[sk,sq] = kT.T @ qT   (lhsT=kT[dk,sk], rhs=qT[dk,sq], K=64)
  avT[dk+1,sq]   = vaug.T @ expT  (lhsT=vaug[sk,65], rhs=expT[sk,sq])
  y[s,e]   = outT.T @ woT      (lhsT=outT[d,s], rhs=woT[d,e])

v2 changes vs baseline (316us):
 - causal trim: diagonal-strip tiles only compute queries >= the tile's
   first key (score-MM N, exp AP, AV N all shrink; ~25% of attention work
   in the diag strips was masked-out waste).
 - masking via PE "tri-add": a -240 strict-upper-triangular constant is
   accumulated into the 128-wide diagonal sub-block of the scores PSUM
   (lhsT=tri, rhs=identity), so exp underflows to ~1e-13 and the DVE mask
   multiply disappears (also removes DVE from the exp->AV critical path).
 - PE fillers (projections/oproj) now run BETWEEN the attention kt loop and
   flush_tail, so the tail AV matmuls never wait on fresh exps.
 - startup: x streams in 512-col slices so proj_v(0) is unblocked sooner;
   warmup uses N=128 matmuls (same HAM-warming busy time, less queue delay).
 - y output is bf16 (halves writeback bytes; host accumulates in fp32).
"""

from contextlib import ExitStack

import numpy as np
import ml_dtypes

import concourse.bass as bass
import concourse.tile as tile
from concourse import bacc, mybir
from concourse._compat import with_exitstack
from concourse.bass_utils import run_bass_kernel_spmd

B, S, D, H = 4, 2048, 1024, 16
DK = D // H          # 64
E = 512              # local features per core (8 heads)
HL = 8               # local heads
NCORES = 8
NDT = D // 128       # 8 d-tiles
NET = E // 128       # 4 e-tiles
NST = S // 128       # 16 s-tiles
NQG = S // 512       # 4 query groups

F32 = mybir.dt.float32
BF16 = mybir.dt.bfloat16
bf16 = ml_dtypes.bfloat16

_compiled = None
last_results = None  # test harness introspection


@with_exitstack
def _mhsa_kernel(ctx: ExitStack, tc: tile.TileContext, y, xT, wqT, wkT, wvT,
                 woT, tri, iden):
    nc = tc.nc

    consts = ctx.enter_context(tc.tile_pool(name="consts", bufs=1))
    ex_pool = ctx.enter_context(tc.tile_pool(name="ex", bufs=8))
    rec_pool = ctx.enter_context(tc.tile_pool(name="rec", bufs=2))
    y_pool = ctx.enter_context(tc.tile_pool(name="ysb", bufs=3))
    ps_pool = ctx.enter_context(tc.tile_pool(name="psmm", bufs=3, space="PSUM"))
    av_pool = ctx.enter_context(tc.tile_pool(name="psav", bufs=2, space="PSUM"))

    def ctile(shape, dt_, tg):
        return consts.tile(shape, dt_, tag=tg, name=tg)

    # ---- persistent SBUF tiles -------------------------------------------
    xT_t = [ctile([128, S], BF16, f"xT{i}") for i in range(NDT)]
    wqT_t = [ctile([128, E], BF16, f"wqT{i}") for i in range(NDT)]
    wkT_t = [ctile([128, E], BF16, f"wkT{i}") for i in range(NDT)]
    wvT_t = [ctile([128, E], BF16, f"wvT{i}") for i in range(NDT)]
    woT_t = [ctile([128, D], BF16, f"woT{i}") for i in range(NET)]
    qT_t = [ctile([128, S], BF16, f"qT{i}") for i in range(NET)]
    kT_t = [ctile([128, S], BF16, f"kT{i}") for i in range(NET)]
    vaug_t = [ctile([128, HL * (DK + 1)], BF16, f"vaug{i}") for i in range(NST)]
    outT_t = [ctile([128, S], BF16, f"outT{i}") for i in range(NET)]
    tri_t = ctile([128, 128], BF16, "tri")
    iden_t = ctile([128, 128], BF16, "iden")

    # ---- input loads, ordered so compute can start ASAP ------------------
    # x streams in 512-col slices on the gpsimd queues (proj_v(0) needs only
    # cols 0:256 of every d-tile); weights on the sync queues in parallel.
    for c in range(4):
        for i in range(NDT):
            nc.gpsimd.dma_start(
                out=xT_t[i][:, c * 512:(c + 1) * 512],
                in_=xT[i * 128:(i + 1) * 128, c * 512:(c + 1) * 512])
    for i in range(NDT):
        nc.sync.dma_start(out=wvT_t[i], in_=wvT[i * 128:(i + 1) * 128, :])
    nc.sync.dma_start(out=tri_t, in_=tri)
    nc.sync.dma_start(out=iden_t, in_=iden)
    for i in range(NDT):
        nc.sync.dma_start(out=wqT_t[i], in_=wqT[i * 128:(i + 1) * 128, :])
        nc.sync.dma_start(out=wkT_t[i], in_=wkT[i * 128:(i + 1) * 128, :])
    for i in range(NET):
        nc.sync.dma_start(out=woT_t[i], in_=woT[i * 128:(i + 1) * 128, :])

    # ---- q/k projections: qT[e,s], kT[e,s] -------------------------------
    def proj_qk(wt, dst, et, scg):
        ps = ps_pool.tile([128, 1024], F32, tag="mm", name="ps")
        for dt_ in range(NDT):
            for hf in range(2):
                s0 = scg * 1024 + hf * 512
                nc.tensor.matmul(
                    ps[:, hf * 512:(hf + 1) * 512],
                    lhsT=wt[dt_][:, et * 128:(et + 1) * 128],
                    rhs=xT_t[dt_][:, s0:s0 + 512],
                    start=(dt_ == 0), stop=(dt_ == NDT - 1),
                )
        nc.vector.tensor_copy(dst[et][:, scg * 1024:(scg + 1) * 1024], ps)

    # ---- v projection -> vaug tiles [128, 8*65] with ones columns --------
    def proj_v(stp):
        ps = ps_pool.tile([128, 1024], F32, tag="mm", name="ps")
        for dt_ in range(NDT):
            for hf in range(2):
                st = 2 * stp + hf
                nc.tensor.matmul(
                    ps[:, hf * 512:(hf + 1) * 512],
                    lhsT=xT_t[dt_][:, st * 128:(st + 1) * 128],
                    rhs=wvT_t[dt_],
                    start=(dt_ == 0), stop=(dt_ == NDT - 1),
                )
        for hf in range(2):
            st = 2 * stp + hf
            nc.vector.memset(vaug_t[st], 1.0)
            # one strided cast: [128, 8, 64] view skips the ones columns
            nc.vector.tensor_copy(
                vaug_t[st].rearrange("p (h c) -> p h c", c=65)[:, :, 0:64],
                ps[:, hf * 512:(hf + 1) * 512].rearrange(
                    "p (h c) -> p h c", c=64),
            )

    # Softmax denominators bounce through DRAM: DVE can only write at
    # 32-aligned base partitions, and SBUF APs cannot have a step-0
    # partition dim (needed for the broadcast) — DRAM APs can do both.
    sums_dram = nc.dram_tensor("sums_bounce", [NQG, HL, 512], F32).ap()
    rec_dram = nc.dram_tensor("rec_bounce", [NQG, HL, 512], BF16).ap()

    # ones2: selector for the final pair's reciprocal broadcast matmul —
    # bc[j, :] = recb2[0, :] for j<64 (head A) and recb2[32, :] for j>=64
    # (head B).  Rows 0/32 because the DVE can only write at 32-aligned
    # partitions; K padded to 64 (a K=33 matmul wedges the exec unit).
    ones2 = ctile([64, 128], BF16, "ones2")
    nc.vector.memset(ones2, 0.0)
    nc.vector.memset(ones2[0:1, 0:64], 1.0)
    nc.vector.memset(ones2[32:33, 64:128], 1.0)

    # PE warm-up: HAM starts throttled at 1.2 GHz and needs ~3.4us of
    # sustained matmul activity to release; burn idle DMA-wait time at
    # kernel start so the first real matmuls run at full clock.  N=128
    # keeps the queue shallow so real work starts as soon as DMAs land.
    warm = ctile([128, 512], BF16, "warm")
    nc.vector.memset(warm, 0.0)
    for _ in range(36):
        wps = ps_pool.tile([128, 512], F32, tag="mm", name="wps")
        nc.tensor.matmul(wps[:, 0:128], lhsT=warm[:, 0:128],
                         rhs=warm[:, 0:128], start=True, stop=True)

    # ---- attention for one (head-pair, query-group) ----------------------
    # Heads hA=2*hp (partitions 0:64) and hB=2*hp+1 (64:128) share each
    # score tile: [:, 0:512]=A, [:, 512:1024]=B for one key tile kt.  The
    # K=64 score matmuls for A and B land on disjoint PE row groups (base
    # partition 0 vs 64) and run concurrently (trace-verified dstart=3ns).
    # Diagonal-strip tiles (kt >= 4*qg) are query-trimmed: only queries
    # >= the tile's first key are computed, and the 128-wide triangular
    # sub-block gets a -240 additive mask folded in via an extra matmul
    # (exp then underflows to ~1e-13: effectively zero, so denominators
    # and AV stay correct with no DVE masking).  outT stays UNNORMALIZED;
    # denominators are collected and normalization is batched per qg so
    # the PE never waits on the reciprocal chain.
    def attn(hp, qg):
        ti = hp
        hA, hB = 2 * hp, 2 * hp + 1
        nk = 4 * qg + 4
        avA = av_pool.tile([65, 512], F32, tag="av", name="avA")
        avB = av_pool.tile([65, 512], F32, tag="av", name="avB")

        def emit_av(kt, ex, off):
            for av, h in ((avA, hA), (avB, hB)):
                nc.tensor.matmul(
                    av[:, off:512],
                    lhsT=vaug_t[kt][:, h * 65:h * 65 + 65],
                    rhs=ex[:, (h & 1) * 512 + off:((h & 1) + 1) * 512],
                    start=(kt == 0), stop=(kt == nk - 1),
                    skip_group_check=True,
                )

        pending = []
        for kt in range(nk):
            j = kt - 4 * qg  # diagonal-strip index (>=0 on the diagonal)
            off = 128 * j if j >= 0 else 0
            diag = j >= 0
            ps = ps_pool.tile([128, 1024], F32, tag="mm", name="ps")
            for po in (0, 64):
                hf = po // 64
                nc.tensor.matmul(
                    ps[:, hf * 512 + off:(hf + 1) * 512],
                    lhsT=kT_t[ti][po:po + 64, kt * 128:(kt + 1) * 128],
                    rhs=qT_t[ti][po:po + 64, qg * 512 + off:(qg + 1) * 512],
                    start=True, stop=not diag,
                    skip_group_check=True,
                )
            if diag:
                for hf in range(2):
                    nc.tensor.matmul(
                        ps[:, hf * 512 + off:hf * 512 + off + 128],
                        lhsT=tri_t, rhs=iden_t,
                        start=False, stop=True,
                        skip_group_check=True,
                    )
            ex = ex_pool.tile([128, 1024], BF16, tag="ex", name="ex")
            if off:
                ps_in = ps.rearrange("p (h q) -> p h q", q=512)[:, :, off:512]
                ex_out = ex.rearrange("p (h q) -> p h q", q=512)[:, :, off:512]
            else:
                ps_in, ex_out = ps, ex
            nc.scalar.activation(out=ex_out, in_=ps_in,
                                 func=mybir.ActivationFunctionType.Exp,
                                 scale=0.125)
            pending.append((kt, ex, off))
            if len(pending) > 2:  # lag 2: AV never waits on a fresh exp
                emit_av(*pending.pop(0))

        # The remaining AV matmuls wait on the freshest exps; returning them
        # as a closure lets the caller slip an independent filler group in
        # front, so the PE chews filler instead of stalling on the ACT.
        def flush_tail():
            for item in pending:
                emit_av(*item)
            _stash(hp, qg, ti, avA, avB)
        return flush_tail

    def _stash(hp, qg, ti, avA, avB):
        # stash unnormalized outputs + denominators; release av quickly
        hA, hB = 2 * hp, 2 * hp + 1
        if qg == NQG - 1 and hp == HL // 2 - 1:
            # final pair: no attention left to hide the DRAM-bounce latency
            # behind, so normalize inline via reciprocal + PE broadcast
            stg2 = rec_pool.tile([64, 512], F32, tag="stg2", name="stg2")
            nc.vector.memset(stg2, 1.0)
            for av, po, row in ((avA, 0, 0), (avB, 64, 32)):
                nc.vector.tensor_copy(
                    outT_t[ti][po:po + 64, qg * 512:(qg + 1) * 512],
                    av[0:64, :])
                nc.vector.tensor_copy(stg2[row:row + 1, :], av[64:65, :])
            rec2 = rec_pool.tile([64, 512], F32, tag="rec2", name="rec2")
            nc.vector.reciprocal_approx_fast(out=rec2, in_=stg2)
            recb2 = rec_pool.tile([64, 512], BF16, tag="recb2", name="recb2")
            nc.vector.tensor_copy(recb2, rec2)
            bc = av_pool.tile([128, 512], F32, tag="av", name="bc")
            nc.tensor.matmul(bc, lhsT=ones2, rhs=recb2, start=True, stop=True)
            for po in (0, 64):
                sl = outT_t[ti][po:po + 64, qg * 512:(qg + 1) * 512]
                nc.vector.tensor_mul(sl, sl, bc[po:po + 64, :])
        else:
            for av, h, po in ((avA, hA, 0), (avB, hB, 64)):
                nc.vector.tensor_copy(
                    outT_t[ti][po:po + 64, qg * 512:(qg + 1) * 512],
                    av[0:64, :])
                stg = rec_pool.tile([1, 512], F32, tag="stg", name="stg",
                                    bufs=4)
                nc.vector.tensor_copy(stg, av[64:65, :])
                nc.sync.dma_start(out=sums_dram[qg, h], in_=stg)

    # ---- batched normalization (DRAM-bounce broadcast) -------------------
    def _norm_heads(qg, heads):
        h0, nh = heads[0], len(heads)
        sums = rec_pool.tile([nh, 512], F32, tag=f"sums{nh}", name="sums")
        nc.sync.dma_start(out=sums, in_=sums_dram[qg, h0:h0 + nh])
        rec = rec_pool.tile([nh, 512], F32, tag=f"rec{nh}", name="rec")
        nc.vector.reciprocal_approx_fast(out=rec, in_=sums)
        recb = rec_pool.tile([nh, 512], BF16, tag=f"recb{nh}", name="recb")
        nc.vector.tensor_copy(recb, rec)
        nc.sync.dma_start(out=rec_dram[qg, h0:h0 + nh], in_=recb)
        for h in heads:
            ti, po = h // 2, 64 * (h % 2)
            # walrus requires SBUF tensor_tensor inputs to share the start
            # partition, so land the broadcast at the same partition range
            bcs = rec_pool.tile([128, 512], BF16, tag="bcs", name="bcs")
            nc.sync.dma_start(
                out=bcs[po:po + 64, :],
                in_=rec_dram[qg, h:h + 1, :].to_broadcast([64, 512]))
            sl = outT_t[ti][po:po + 64, qg * 512:(qg + 1) * 512]
            nc.vector.tensor_mul(sl, sl, bcs[po:po + 64, :])

    def normalize(qg):
        _norm_heads(qg, list(range(HL)))

    def normalize_pair(qg, hp):
        _norm_heads(qg, [2 * hp, 2 * hp + 1])

    # ---- o-projection: y[s,:] partial ------------------------------------
    def oproj(st):
        ps = ps_pool.tile([128, 1024], F32, tag="mm", name="ps")
        for dt_ in range(NET):
            for hf in range(2):
                nc.tensor.matmul(
                    ps[:, hf * 512:(hf + 1) * 512],
                    lhsT=outT_t[dt_][:, st * 128:(st + 1) * 128],
                    rhs=woT_t[dt_][:, hf * 512:(hf + 1) * 512],
                    start=(dt_ == 0), stop=(dt_ == NET - 1),
                )
        ysb = y_pool.tile([128, 1024], BF16, tag="ysb", name="ysb")
        # split copy+DMA per half so the writeback starts earlier; alternate
        # queues so the tail drain parallelizes
        for hf in range(2):
            nc.vector.tensor_copy(ysb[:, hf * 512:(hf + 1) * 512],
                                  ps[:, hf * 512:(hf + 1) * 512])
            q = nc.sync if hf == 0 else nc.gpsimd
            q.dma_start(
                out=y[st * 128:(st + 1) * 128, hf * 512:(hf + 1) * 512],
                in_=ysb[:, hf * 512:(hf + 1) * 512])

    # ---- program order ----------------------------------------------------
    # Attention is ACT(exp)-bound per kt, so start it as soon as its first
    # dependencies exist (qg0 needs only q/k et0 cols 0:512 and v st0..3)
    # and drip the remaining PE-only projection work as filler INSIDE each
    # head-pair slot (between the kt loop and flush_tail), where it soaks
    # up the PE's wait-on-exp slack without stalling the tail AVs.
    def qk_pair(et, scg):
        proj_qk(wqT_t, qT_t, et, scg)
        proj_qk(wkT_t, kT_t, et, scg)

    proj_v(0)
    proj_v(1)
    qk_pair(0, 0)

    # pre-fillers run between the kt loop and flush_tail; post-fillers run
    # after flush_tail.  normalize_pair(qg, hp) MUST stay post (its
    # sums_bounce RAW dep on this slot's stash is ordered only by
    # sync-queue program order).
    pre = {
        (0, 0): [lambda: qk_pair(1, 0), lambda: proj_v(2)],
        (0, 1): [lambda: qk_pair(2, 0), lambda: proj_v(3)],
        (0, 2): [lambda: qk_pair(3, 0), lambda: proj_v(4)],
        (0, 3): [lambda: proj_v(5)],
        (1, 0): [lambda: proj_v(6)],
        (1, 1): [lambda: proj_v(7)],
        (1, 2): [lambda: qk_pair(0, 1)],
        (1, 3): [lambda: qk_pair(1, 1)],
        (2, 0): [lambda: qk_pair(2, 1), lambda: normalize(0)],
        (2, 1): [lambda: qk_pair(3, 1), lambda: oproj(0)],
        (2, 2): [lambda: normalize(1), lambda: oproj(1)],
        (2, 3): [lambda: oproj(2), lambda: oproj(3)],
        (3, 0): [lambda: normalize(2), lambda: oproj(4)],
        (3, 1): [lambda: oproj(5), lambda: oproj(6)],
        (3, 2): [lambda: oproj(7), lambda: oproj(8)],
        (3, 3): [lambda: oproj(9), lambda: oproj(10), lambda: oproj(11)],
    }
    post = {
        (3, 0): [lambda: normalize_pair(3, 0)],
        (3, 1): [lambda: normalize_pair(3, 1)],
        (3, 2): [lambda: normalize_pair(3, 2)],
    }
    for qg in range(NQG):
        for hp in range(HL // 2):
            flush_tail = attn(hp, qg)
            for f in pre.get((qg, hp), []):
                f()
            flush_tail()
            for f in post.get((qg, hp), []):
                f()
    for st in range(4 * (NQG - 1), 4 * NQG):
        oproj(st)


def _build():
    nc = bacc.Bacc("TRN2", target_bir_lowering=False, debug=False,
                   num_devices=NCORES)
    xT = nc.dram_tensor("xT", [D, S], BF16, kind="ExternalInput").ap()
    wqT = nc.dram_tensor("wqT", [D, E], BF16, kind="ExternalInput").ap()
    wkT = nc.dram_tensor("wkT", [D, E], BF16, kind="ExternalInput").ap()
    wvT = nc.dram_tensor("wvT", [D, E], BF16, kind="ExternalInput").ap()
    woT = nc.dram_tensor("woT", [E, D], BF16, kind="ExternalInput").ap()
    tri = nc.dram_tensor("tri", [128, 128], BF16, kind="ExternalInput").ap()
    iden = nc.dram_tensor("iden", [128, 128], BF16, kind="ExternalInput").ap()
    y = nc.dram_tensor("y", [S, D], BF16, kind="ExternalOutput").ap()
    with tile.TileContext(nc) as tc:
        _mhsa_kernel(tc, y, xT, wqT, wkT, wvT, woT, tri, iden)
    nc.compile()
    return nc


def get_compiled():
    global _compiled
    if _compiled is None:
        _compiled = _build()
    return _compiled


def _make_consts():
    # tri: strict upper-triangular -240 additive mask (row j < col k means
    # query j attends-not to key k within the 128-wide diagonal sub-block);
    # after the 1/8 exp scale this is exp(-30+s) ~ 1e-13: effectively zero.
    tri = (np.triu(np.ones((128, 128), dtype=np.float32), 1) * -240.0)
    iden = np.eye(128, dtype=np.float32)
    return tri.astype(bf16), iden.astype(bf16)


def kernel(**inputs):
    global last_results
    x = np.asarray(inputs["in_features"], dtype=np.float32)
    w_q = np.asarray(inputs["w_q"], dtype=np.float32)
    w_k = np.asarray(inputs["w_k"], dtype=np.float32)
    w_v = np.asarray(inputs["w_v"], dtype=np.float32)
    w_o = np.asarray(inputs["w_o"], dtype=np.float32)

    nc = get_compiled()
    tri, iden = _make_consts()
    in_maps = []
    for c in range(NCORES):
        b, hg = divmod(c, 2)
        es = slice(hg * E, (hg + 1) * E)
        in_maps.append({
            "xT": x[b].T.astype(bf16),
            "wqT": w_q[es, :].T.astype(bf16),
            "wkT": w_k[es, :].T.astype(bf16),
            "wvT": w_v[es, :].T.astype(bf16),
            "woT": w_o[:, es].T.astype(bf16),
            "tri": tri,
            "iden": iden,
        })
    res = run_bass_kernel_spmd(nc, in_maps, list(range(NCORES)))
    last_results = res
    y = np.zeros((B, S, D), dtype=np.float32)
    for c in range(NCORES):
        y[c // 2] += np.asarray(res.results[c]["y"], dtype=np.float32)
    return y


# revision 4
# speedup vs baseline: 5309.1628x; 5309.1628x over previous
"""Multi-head self-attention (causal) Trainium2 Bass/Tile kernel, 8-way SPMD.

Sharding: data-parallel over batch (4) x tensor-parallel over heads (2 groups
of 8 heads).  Core c handles batch c//2, head-group c%2.  Each core computes
q/k/v projections for its 512 local features, causal attention for its 8
heads, and a partial o-projection (contraction over its 512 features of the
attention output) giving a full-shape [S, D] partial that the host sums per
batch pair.

All matmul operands are bf16 (fp32 PSUM accumulation); softmax runs without
max-subtraction (scores ~ N(0,1) after the 1/8 scale, no overflow risk), with
exp on the scalar engine and the row-sum folded into the AV matmul via a ones
column appended to V.  Host pre-transposes inputs so no on-chip transposes
are needed:
  qT[e,s]  = wqT.T @ xT        (lhsT=wqT[d,e], rhs=xT[d,s])
  scoresT[sk,sq] = kT.T @ qT   (lhsT=kT[dk,sk], rhs=qT[dk,sq], K=64)
  avT[dk+1,sq]   = vaug.T @ expT  (lhsT=vaug[sk,65], rhs=expT[sk,sq])
  y[s,e]   = outT.T @ woT      (lhsT=outT[d,s], rhs=woT[d,e])

v3 vs v2 (307.5us):
 - masking back on DVE (PE tri-add cost more PE than it saved), but only on
   the trimmed 128-wide diagonal sub-block ([128,2,128] strided mul, ~194ns).
 - filler projection/oproj units are injected INSIDE each attention kt loop
   at spread points, so the PE never stalls on the exp stream's PSUM-buffer
   recycling (the v2 per-slot ~1.1-1.4us gaps).
 - query groups visit in order [0,1,3,2]: the ACT-heavy qg3 slots run while
   oproj filler still exists; qg2 (last) uses per-pair normalization and the
   final slot normalizes inline.
 - warmup back to 16 x N=512 matmuls (N=128 failed to trip the HAM window).
"""

from contextlib import ExitStack

import numpy as np
import ml_dtypes

import concourse.bass as bass
import concourse.tile as tile
from concourse import bacc, mybir
from concourse._compat import with_exitstack
from concourse.bass_utils import run_bass_kernel_spmd

B, S, D, H = 4, 2048, 1024, 16
DK = D // H          # 64
E = 512              # local features per core (8 heads)
HL = 8               # local heads
NCORES = 8
NDT = D // 128       # 8 d-tiles
NET = E // 128       # 4 e-tiles
NST = S // 128       # 16 s-tiles
NQG = S // 512       # 4 query groups

QG_ORDER = [0, 1, 3, 2]          # visit order; last visited gets inline norm
LAST_QG = QG_ORDER[-1]

F32 = mybir.dt.float32
BF16 = mybir.dt.bfloat16
bf16 = ml_dtypes.bfloat16

_compiled = None
last_results = None  # test harness introspection


@with_exitstack
def _mhsa_kernel(ctx: ExitStack, tc: tile.TileContext, y, xT, wqT, wkT, wvT,
                 woT, m01):
    nc = tc.nc

    consts = ctx.enter_context(tc.tile_pool(name="consts", bufs=1))
    ex_pool = ctx.enter_context(tc.tile_pool(name="ex", bufs=8))
    rec_pool = ctx.enter_context(tc.tile_pool(name="rec", bufs=2))
    y_pool = ctx.enter_context(tc.tile_pool(name="ysb", bufs=3))
    ps_pool = ctx.enter_context(tc.tile_pool(name="psmm", bufs=3, space="PSUM"))
    av_pool = ctx.enter_context(tc.tile_pool(name="psav", bufs=2, space="PSUM"))

    def ctile(shape, dt_, tg):
        return consts.tile(shape, dt_, tag=tg, name=tg)

    # ---- persistent SBUF tiles -------------------------------------------
    xT_t = [ctile([128, S], BF16, f"xT{i}") for i in range(NDT)]
    wqT_t = [ctile([128, E], BF16, f"wqT{i}") for i in range(NDT)]
    wkT_t = [ctile([128, E], BF16, f"wkT{i}") for i in range(NDT)]
    wvT_t = [ctile([128, E], BF16, f"wvT{i}") for i in range(NDT)]
    woT_t = [ctile([128, D], BF16, f"woT{i}") for i in range(NET)]
    qT_t = [ctile([128, S], BF16, f"qT{i}") for i in range(NET)]
    kT_t = [ctile([128, S], BF16, f"kT{i}") for i in range(NET)]
    vaug_t = [ctile([128, HL * (DK + 1)], BF16, f"vaug{i}") for i in range(NST)]
    outT_t = [ctile([128, S], BF16, f"outT{i}") for i in range(NET)]
    m01_t = ctile([128, 256], BF16, "m01")

    # ---- input loads, ordered so compute can start ASAP ------------------
    for c in range(4):
        for i in range(NDT):
            nc.gpsimd.dma_start(
                out=xT_t[i][:, c * 512:(c + 1) * 512],
                in_=xT[i * 128:(i + 1) * 128, c * 512:(c + 1) * 512])
    for i in range(NDT):
        nc.sync.dma_start(out=wvT_t[i], in_=wvT[i * 128:(i + 1) * 128, :])
    nc.sync.dma_start(out=m01_t, in_=m01)
    for i in range(NDT):
        nc.sync.dma_start(out=wqT_t[i], in_=wqT[i * 128:(i + 1) * 128, :])
        nc.sync.dma_start(out=wkT_t[i], in_=wkT[i * 128:(i + 1) * 128, :])
    for i in range(NET):
        nc.sync.dma_start(out=woT_t[i], in_=woT[i * 128:(i + 1) * 128, :])

    # ---- q/k projections: qT[e,s], kT[e,s] -------------------------------
    def proj_qk(wt, dst, et, scg):
        ps = ps_pool.tile([128, 1024], F32, tag="mm", name="ps")
        for dt_ in range(NDT):
            for hf in range(2):
                s0 = scg * 1024 + hf * 512
                nc.tensor.matmul(
                    ps[:, hf * 512:(hf + 1) * 512],
                    lhsT=wt[dt_][:, et * 128:(et + 1) * 128],
                    rhs=xT_t[dt_][:, s0:s0 + 512],
                    start=(dt_ == 0), stop=(dt_ == NDT - 1),
                )
        nc.vector.tensor_copy(dst[et][:, scg * 1024:(scg + 1) * 1024], ps)

    # ---- v projection -> vaug tiles [128, 8*65] with ones columns --------
    def proj_v(stp):
        ps = ps_pool.tile([128, 1024], F32, tag="mm", name="ps")
        for dt_ in range(NDT):
            for hf in range(2):
                st = 2 * stp + hf
                nc.tensor.matmul(
                    ps[:, hf * 512:(hf + 1) * 512],
                    lhsT=xT_t[dt_][:, st * 128:(st + 1) * 128],
                    rhs=wvT_t[dt_],
                    start=(dt_ == 0), stop=(dt_ == NDT - 1),
                )
        for hf in range(2):
            st = 2 * stp + hf
            nc.vector.memset(vaug_t[st], 1.0)
            nc.vector.tensor_copy(
                vaug_t[st].rearrange("p (h c) -> p h c", c=65)[:, :, 0:64],
                ps[:, hf * 512:(hf + 1) * 512].rearrange(
                    "p (h c) -> p h c", c=64),
            )

    sums_dram = nc.dram_tensor("sums_bounce", [NQG, HL, 512], F32).ap()
    rec_dram = nc.dram_tensor("rec_bounce", [NQG, HL, 512], BF16).ap()

    # ones2: selector for the final pair's reciprocal broadcast matmul
    ones2 = ctile([64, 128], BF16, "ones2")
    nc.vector.memset(ones2, 0.0)
    nc.vector.memset(ones2[0:1, 0:64], 1.0)
    nc.vector.memset(ones2[32:33, 64:128], 1.0)

    # PE warm-up: ~6.8us of solid matmul streaming releases the HAM throttle
    warm = ctile([128, 512], BF16, "warm")
    nc.vector.memset(warm, 0.0)
    for _ in range(16):
        wps = ps_pool.tile([128, 512], F32, tag="mm", name="wps")
        nc.tensor.matmul(wps, lhsT=warm[:, 0:128], rhs=warm,
                         start=True, stop=True)

    # ---- attention for one (head-pair, query-group) ----------------------
    def attn(hp, qg, units):
        ti = hp
        hA, hB = 2 * hp, 2 * hp + 1
        nk = 4 * qg + 4
        avA = av_pool.tile([65, 512], F32, tag="av", name="avA")
        avB = av_pool.tile([65, 512], F32, tag="av", name="avB")

        # spread filler units across the kt loop at computed points
        n_u = len(units)
        inject = {}
        for i in range(n_u):
            pt = (i + 1) * nk // (n_u + 1)
            inject.setdefault(pt, []).append(units[i])

        def emit_av(kt, ex, off):
            for av, h in ((avA, hA), (avB, hB)):
                nc.tensor.matmul(
                    av[:, off:512],
                    lhsT=vaug_t[kt][:, h * 65:h * 65 + 65],
                    rhs=ex[:, (h & 1) * 512 + off:((h & 1) + 1) * 512],
                    start=(kt == 0), stop=(kt == nk - 1),
                    skip_group_check=True,
                )

        pending = []
        for kt in range(nk):
            j = kt - 4 * qg
            off = 128 * j if j >= 0 else 0
            diag = j >= 0
            ps = ps_pool.tile([128, 1024], F32, tag="mm", name="ps")
            for po in (0, 64):
                hf = po // 64
                nc.tensor.matmul(
                    ps[:, hf * 512 + off:(hf + 1) * 512],
                    lhsT=kT_t[ti][po:po + 64, kt * 128:(kt + 1) * 128],
                    rhs=qT_t[ti][po:po + 64, qg * 512 + off:(qg + 1) * 512],
                    start=True, stop=True,
                )
            ex = ex_pool.tile([128, 1024], BF16, tag="ex", name="ex")
            if off:
                ps_in = ps.rearrange("p (h q) -> p h q", q=512)[:, :, off:512]
                ex_out = ex.rearrange("p (h q) -> p h q", q=512)[:, :, off:512]
            else:
                ps_in, ex_out = ps, ex
            nc.scalar.activation(out=ex_out, in_=ps_in,
                                 func=mybir.ActivationFunctionType.Exp,
                                 scale=0.125)
            if diag:  # 0/1 mask on the 128-wide triangular sub-block only
                exm = ex.rearrange("p (h q) -> p h q", q=512)[:, :,
                                                             off:off + 128]
                nc.vector.tensor_mul(
                    exm, exm, m01_t.rearrange("p (h q) -> p h q", q=128))
            pending.append((kt, ex, off))
            if len(pending) > 2:  # lag 2: AV never waits on a fresh exp
                emit_av(*pending.pop(0))
            for u in inject.get(kt, []):
                u()

        def flush_tail():
            for item in pending:
                emit_av(*item)
            _stash(hp, qg, ti, avA, avB)
        return flush_tail

    def _stash(hp, qg, ti, avA, avB):
        hA, hB = 2 * hp, 2 * hp + 1
        if qg == LAST_QG and hp == HL // 2 - 1:
            # final slot: normalize inline via reciprocal + PE broadcast
            stg2 = rec_pool.tile([64, 512], F32, tag="stg2", name="stg2")
            nc.vector.memset(stg2, 1.0)
            for av, po, row in ((avA, 0, 0), (avB, 64, 32)):
                nc.vector.tensor_copy(
                    outT_t[ti][po:po + 64, qg * 512:(qg + 1) * 512],
                    av[0:64, :])
                nc.vector.tensor_copy(stg2[row:row + 1, :], av[64:65, :])
            rec2 = rec_pool.tile([64, 512], F32, tag="rec2", name="rec2")
            nc.vector.reciprocal_approx_fast(out=rec2, in_=stg2)
            recb2 = rec_pool.tile([64, 512], BF16, tag="recb2", name="recb2")
            nc.vector.tensor_copy(recb2, rec2)
            bc = av_pool.tile([128, 512], F32, tag="av", name="bc")
            nc.tensor.matmul(bc, lhsT=ones2, rhs=recb2, start=True, stop=True)
            for po in (0, 64):
                sl = outT_t[ti][po:po + 64, qg * 512:(qg + 1) * 512]
                nc.vector.tensor_mul(sl, sl, bc[po:po + 64, :])
        else:
            for av, h, po in ((avA, hA, 0), (avB, hB, 64)):
                nc.vector.tensor_copy(
                    outT_t[ti][po:po + 64, qg * 512:(qg + 1) * 512],
                    av[0:64, :])
                stg = rec_pool.tile([1, 512], F32, tag="stg", name="stg",
                                    bufs=4)
                nc.vector.tensor_copy(stg, av[64:65, :])
                nc.sync.dma_start(out=sums_dram[qg, h], in_=stg)

    # ---- batched normalization (DRAM-bounce broadcast) -------------------
    def _norm_heads(qg, heads):
        h0, nh = heads[0], len(heads)
        sums = rec_pool.tile([nh, 512], F32, tag=f"sums{nh}", name="sums")
        nc.sync.dma_start(out=sums, in_=sums_dram[qg, h0:h0 + nh])
        rec = rec_pool.tile([nh, 512], F32, tag=f"rec{nh}", name="rec")
        nc.vector.reciprocal_approx_fast(out=rec, in_=sums)
        recb = rec_pool.tile([nh, 512], BF16, tag=f"recb{nh}", name="recb")
        nc.vector.tensor_copy(recb, rec)
        nc.sync.dma_start(out=rec_dram[qg, h0:h0 + nh], in_=recb)
        for h in heads:
            ti, po = h // 2, 64 * (h % 2)
            bcs = rec_pool.tile([128, 512], BF16, tag="bcs", name="bcs")
            nc.sync.dma_start(
                out=bcs[po:po + 64, :],
                in_=rec_dram[qg, h:h + 1, :].to_broadcast([64, 512]))
            sl = outT_t[ti][po:po + 64, qg * 512:(qg + 1) * 512]
            nc.vector.tensor_mul(sl, sl, bcs[po:po + 64, :])

    def normalize(qg):
        _norm_heads(qg, list(range(HL)))

    def normalize_pair(qg, hp):
        _norm_heads(qg, [2 * hp, 2 * hp + 1])

    # ---- o-projection: y[s,:] partial ------------------------------------
    def oproj(st):
        ps = ps_pool.tile([128, 1024], F32, tag="mm", name="ps")
        for dt_ in range(NET):
            for hf in range(2):
                nc.tensor.matmul(
                    ps[:, hf * 512:(hf + 1) * 512],
                    lhsT=outT_t[dt_][:, st * 128:(st + 1) * 128],
                    rhs=woT_t[dt_][:, hf * 512:(hf + 1) * 512],
                    start=(dt_ == 0), stop=(dt_ == NET - 1),
                )
        ysb = y_pool.tile([128, 1024], BF16, tag="ysb", name="ysb")
        for hf in range(2):
            nc.vector.tensor_copy(ysb[:, hf * 512:(hf + 1) * 512],
                                  ps[:, hf * 512:(hf + 1) * 512])
            q = nc.sync if hf == 0 else nc.gpsimd
            q.dma_start(
                out=y[st * 128:(st + 1) * 128, hf * 512:(hf + 1) * 512],
                in_=ysb[:, hf * 512:(hf + 1) * 512])

    # ---- program order ----------------------------------------------------
    def qkQ(et, scg):
        return lambda: proj_qk(wqT_t, qT_t, et, scg)

    def qkK(et, scg):
        return lambda: proj_qk(wkT_t, kT_t, et, scg)

    def V(stp):
        return lambda: proj_v(stp)

    def O(st):
        return lambda: oproj(st)

    def N_(qg):
        return lambda: normalize(qg)

    proj_v(0)
    proj_v(1)
    proj_qk(wqT_t, qT_t, 0, 0)
    proj_qk(wkT_t, kT_t, 0, 0)

    # filler units injected inside each slot's kt loop (slot = (qg, hp) in
    # QG_ORDER-major, hp-minor order, s = visit index 0..15).  Deadlines:
    # qk(et,scg) before the first slot of a qg using scg that reads et;
    # v(stp) before any slot whose kt loop reaches st=2*stp; o(st) after
    # normalize of st's qg; normalize(qg) after all four qg stashes.
    fillers = {
        (0, 0): [qkQ(1, 0), qkK(1, 0)],
        (0, 1): [qkQ(2, 0), qkK(2, 0)],
        (0, 2): [qkQ(3, 0), qkK(3, 0)],
        (0, 3): [V(2), V(3)],
        (1, 0): [V(4), qkQ(0, 1), N_(0)],
        (1, 1): [V(5), qkK(0, 1)],
        (1, 2): [V(6), qkQ(1, 1)],
        (1, 3): [V(7), qkK(1, 1)],
        (3, 0): [qkQ(2, 1), qkK(2, 1), N_(1), O(0)],
        (3, 1): [qkQ(3, 1), qkK(3, 1), O(1)],
        (3, 2): [O(2), O(3), O(4)],
        (3, 3): [O(5), O(6), O(7)],
        (2, 0): [N_(3), O(12), O(13)],
        (2, 1): [O(14), O(15)],
        (2, 2): [],
        (2, 3): [],
    }
    post = {
        (2, 0): [lambda: normalize_pair(2, 0)],
        (2, 1): [lambda: normalize_pair(2, 1)],
        (2, 2): [lambda: normalize_pair(2, 2)],
    }
    for qg in QG_ORDER:
        for hp in range(HL // 2):
            flush_tail = attn(hp, qg, fillers.get((qg, hp), []))
            flush_tail()
            for f in post.get((qg, hp), []):
                f()
    for st in range(8, 12):  # qg2 (visited last) o-projections
        oproj(st)


def _build():
    nc = bacc.Bacc("TRN2", target_bir_lowering=False, debug=False,
                   num_devices=NCORES)
    xT = nc.dram_tensor("xT", [D, S], BF16, kind="ExternalInput").ap()
    wqT = nc.dram_tensor("wqT", [D, E], BF16, kind="ExternalInput").ap()
    wkT = nc.dram_tensor("wkT", [D, E], BF16, kind="ExternalInput").ap()
    wvT = nc.dram_tensor("wvT", [D, E], BF16, kind="ExternalInput").ap()
    woT = nc.dram_tensor("woT", [E, D], BF16, kind="ExternalInput").ap()
    m01 = nc.dram_tensor("m01", [128, 256], BF16, kind="ExternalInput").ap()
    y = nc.dram_tensor("y", [S, D], BF16, kind="ExternalOutput").ap()
    with tile.TileContext(nc) as tc:
        _mhsa_kernel(tc, y, xT, wqT, wkT, wvT, woT, m01)
    nc.compile()
    return nc


def get_compiled():
    global _compiled
    if _compiled is None:
        _compiled = _build()
    return _compiled


def _make_consts():
    # m01[k, qq] = 1 iff query qq >= key k within the 128-wide diagonal
    # sub-block; duplicated for the two packed heads.
    tri = np.triu(np.ones((128, 128), dtype=np.float32))
    m01 = np.concatenate([tri, tri], axis=1)
    return m01.astype(bf16)


def kernel(**inputs):
    global last_results
    x = np.asarray(inputs["in_features"], dtype=np.float32)
    w_q = np.asarray(inputs["w_q"], dtype=np.float32)
    w_k = np.asarray(inputs["w_k"], dtype=np.float32)
    w_v = np.asarray(inputs["w_v"], dtype=np.float32)
    w_o = np.asarray(inputs["w_o"], dtype=np.float32)

    nc = get_compiled()
    m01 = _make_consts()
    in_maps = []
    for c in range(NCORES):
        b, hg = divmod(c, 2)
        es = slice(hg * E, (hg + 1) * E)
        in_maps.append({
            "xT": x[b].T.astype(bf16),
            "wqT": w_q[es, :].T.astype(bf16),
            "wkT": w_k[es, :].T.astype(bf16),
            "wvT": w_v[es, :].T.astype(bf16),
            "woT": w_o[:, es].T.astype(bf16),
            "m01": m01,
        })
    res = run_bass_kernel_spmd(nc, in_maps, list(range(NCORES)))
    last_results = res
    y = np.zeros((B, S, D), dtype=np.float32)
    for c in range(NCORES):
        y[c // 2] += np.asarray(res.results[c]["y"], dtype=np.float32)
    return y


# revision 10
# speedup vs baseline: 6661.5352x; 1.2547x over previous
"""Multi-head self-attention (causal) Trainium2 Bass/Tile kernel, 8-way SPMD.

Sharding: data-parallel over batch (4) x tensor-parallel over heads (2 groups
of 8 heads).  Core c handles batch c//2, head-group c%2.  Each core computes
q/k/v projections for its 512 local features, causal attention for its 8
heads, and a partial o-projection (contraction over its 512 features of the
attention output) giving a full-shape [S, D] partial that the host sums per
batch pair.

All matmul operands are bf16 (fp32 PSUM accumulation); softmax runs without
max-subtraction (scores ~ N(0,1) after the 1/8 scale, no overflow risk), with
exp on the scalar engine and the row-sum folded into the AV matmul via a ones
column appended to V.  Host pre-transposes inputs so no on-chip transposes
are needed:
  qT[e,s]  = wqT.T @ xT        (lhsT=wqT[d,e], rhs=xT[d,s])
  scoresT[sk,sq] = kT.T @ qT   (lhsT=kT[dk,sk], rhs=qT[dk,sq], K=64)
  avT[dk+1,sq]   = vaug.T @ expT  (lhsT=vaug[sk,65], rhs=expT[sk,sq])
  y[s,e]   = outT.T @ woT      (lhsT=outT[d,s], rhs=woT[d,e])

v3 vs v2 (307.5us):
 - masking back on DVE (PE tri-add cost more PE than it saved), but only on
   the trimmed 128-wide diagonal sub-block ([128,2,128] strided mul, ~194ns).
 - filler projection/oproj units are injected INSIDE each attention kt loop
   at spread points, so the PE never stalls on the exp stream's PSUM-buffer
   recycling (the v2 per-slot ~1.1-1.4us gaps).
 - query groups visit in order [0,1,3,2]: the ACT-heavy qg3 slots run while
   oproj filler still exists; qg2 (last) uses per-pair normalization and the
   final slot normalizes inline.
 - warmup back to 16 x N=512 matmuls (N=128 failed to trip the HAM window).
"""

from contextlib import ExitStack

import numpy as np
import ml_dtypes

import concourse.bass as bass
import concourse.tile as tile
from concourse import bacc, mybir
from concourse._compat import with_exitstack
from concourse.bass_utils import run_bass_kernel_spmd

B, S, D, H = 4, 2048, 1024, 16
DK = D // H          # 64
E = 512              # local features per core (8 heads)
HL = 8               # local heads
NCORES = 8
NDT = D // 128       # 8 d-tiles
NET = E // 128       # 4 e-tiles
NST = S // 128       # 16 s-tiles
NQG = S // 512       # 4 query groups

QG_ORDER = [0, 1, 3, 2]          # visit order; last visited gets inline norm
LAST_QG = QG_ORDER[-1]

F32 = mybir.dt.float32
BF16 = mybir.dt.bfloat16
bf16 = ml_dtypes.bfloat16

_compiled = None
last_results = None  # test harness introspection


@with_exitstack
def _mhsa_kernel(ctx: ExitStack, tc: tile.TileContext, y, xT, wqT, wkT, wvT,
                 woT, m01):
    nc = tc.nc

    consts = ctx.enter_context(tc.tile_pool(name="consts", bufs=1))
    ex_pool = ctx.enter_context(tc.tile_pool(name="ex", bufs=8))
    rec_pool = ctx.enter_context(tc.tile_pool(name="rec", bufs=2))
    y_pool = ctx.enter_context(tc.tile_pool(name="ysb", bufs=3))
    ps_pool = ctx.enter_context(tc.tile_pool(name="psmm", bufs=2, space="PSUM"))
    fl_pool = ctx.enter_context(tc.tile_pool(name="psfl", bufs=2, space="PSUM"))
    av_pool = ctx.enter_context(tc.tile_pool(name="psav", bufs=2, space="PSUM"))

    def ctile(shape, dt_, tg):
        return consts.tile(shape, dt_, tag=tg, name=tg)

    # ---- persistent SBUF tiles -------------------------------------------
    xT_t = [ctile([128, S], BF16, f"xT{i}") for i in range(NDT)]
    wqT_t = [ctile([128, E], BF16, f"wqT{i}") for i in range(NDT)]
    wkT_t = [ctile([128, E], BF16, f"wkT{i}") for i in range(NDT)]
    wvT_t = [ctile([128, E], BF16, f"wvT{i}") for i in range(NDT)]
    woT_t = [ctile([128, D], BF16, f"woT{i}") for i in range(NET)]
    qT_t = [ctile([128, S], BF16, f"qT{i}") for i in range(NET)]
    kT_t = [ctile([128, S], BF16, f"kT{i}") for i in range(NET)]
    vaug_t = [ctile([128, HL * (DK + 1)], BF16, f"vaug{i}") for i in range(NST)]
    outT_t = [ctile([128, S], BF16, f"outT{i}") for i in range(NET)]
    m01_t = ctile([128, 256], BF16, "m01")

    # ---- input loads, ordered so compute can start ASAP ------------------
    for c in range(4):
        for i in range(NDT):
            nc.gpsimd.dma_start(
                out=xT_t[i][:, c * 512:(c + 1) * 512],
                in_=xT[i * 128:(i + 1) * 128, c * 512:(c + 1) * 512])
    for i in range(NDT):
        nc.sync.dma_start(out=wvT_t[i], in_=wvT[i * 128:(i + 1) * 128, :])
    nc.sync.dma_start(out=m01_t, in_=m01)
    for i in range(NDT):
        nc.sync.dma_start(out=wqT_t[i], in_=wqT[i * 128:(i + 1) * 128, :])
        nc.sync.dma_start(out=wkT_t[i], in_=wkT[i * 128:(i + 1) * 128, :])
    for i in range(NET):
        nc.sync.dma_start(out=woT_t[i], in_=woT[i * 128:(i + 1) * 128, :])

    # ---- q/k projections: qT[e,s], kT[e,s] -------------------------------
    # Fillers use their own [128,512] PSUM pool so a filler matmul never
    # waits on a score tile's exp or another filler's DVE drain.
    def proj_qk_h(wt, dst, et, scg, hf):
        ps = fl_pool.tile([128, 512], F32, tag="fl", name="fps")
        s0 = scg * 1024 + hf * 512
        for dt_ in range(NDT):
            nc.tensor.matmul(
                ps,
                lhsT=wt[dt_][:, et * 128:(et + 1) * 128],
                rhs=xT_t[dt_][:, s0:s0 + 512],
                start=(dt_ == 0), stop=(dt_ == NDT - 1),
            )
        nc.vector.tensor_copy(dst[et][:, s0:s0 + 512], ps)

    def proj_qk(wt, dst, et, scg):
        proj_qk_h(wt, dst, et, scg, 0)
        proj_qk_h(wt, dst, et, scg, 1)

    # ---- v projection -> vaug tiles [128, 8*65] with ones columns --------
    def proj_v_st(st):
        ps = fl_pool.tile([128, 512], F32, tag="fl", name="fps")
        for dt_ in range(NDT):
            nc.tensor.matmul(
                ps,
                lhsT=xT_t[dt_][:, st * 128:(st + 1) * 128],
                rhs=wvT_t[dt_],
                start=(dt_ == 0), stop=(dt_ == NDT - 1),
            )
        nc.vector.memset(vaug_t[st], 1.0)
        nc.vector.tensor_copy(
            vaug_t[st].rearrange("p (h c) -> p h c", c=65)[:, :, 0:64],
            ps.rearrange("p (h c) -> p h c", c=64),
        )

    def proj_v(stp):
        proj_v_st(2 * stp)
        proj_v_st(2 * stp + 1)

    sums_dram = nc.dram_tensor("sums_bounce", [NQG, HL, 512], F32).ap()
    rec_dram = nc.dram_tensor("rec_bounce", [NQG, HL, 512], BF16).ap()

    # ones2: selector for the final pair's reciprocal broadcast matmul
    ones2 = ctile([64, 128], BF16, "ones2")
    nc.vector.memset(ones2, 0.0)
    nc.vector.memset(ones2[0:1, 0:64], 1.0)
    nc.vector.memset(ones2[32:33, 64:128], 1.0)

    # PE warm-up: ~6.8us of solid matmul streaming releases the HAM throttle
    warm = ctile([128, 512], BF16, "warm")
    nc.vector.memset(warm, 0.0)
    for _ in range(16):
        wps = ps_pool.tile([128, 512], F32, tag="mm", name="wps")
        nc.tensor.matmul(wps, lhsT=warm[:, 0:128], rhs=warm,
                         start=True, stop=True)

    # ---- attention for one (head-pair, query-group) ----------------------
    def attn(hp, qg, units):
        ti = hp
        hA, hB = 2 * hp, 2 * hp + 1
        nk = 4 * qg + 4
        avA = av_pool.tile([65, 512], F32, tag="av", name="avA")
        avB = av_pool.tile([65, 512], F32, tag="av", name="avB")

        # spread filler units across the kt loop, end-biased so the last
        # kts and the tail AVs also have PE work queued behind them
        n_u = len(units)
        inject = {}
        for i in range(n_u):
            pt = (i + 1) * (nk - 1) // n_u if n_u else 0
            inject.setdefault(pt, []).append(units[i])

        def emit_av(kt, ex, off):
            for av, h in ((avA, hA), (avB, hB)):
                nc.tensor.matmul(
                    av[:, off:512],
                    lhsT=vaug_t[kt][:, h * 65:h * 65 + 65],
                    rhs=ex[:, (h & 1) * 512 + off:((h & 1) + 1) * 512],
                    start=(kt == 0), stop=(kt == nk - 1),
                    skip_group_check=True,
                )

        pending = []
        for kt in range(nk):
            j = kt - 4 * qg
            off = 128 * j if j >= 0 else 0
            diag = j >= 0
            ps = ps_pool.tile([128, 1024], F32, tag="mm", name="ps")
            for po in (0, 64):
                hf = po // 64
                nc.tensor.matmul(
                    ps[:, hf * 512 + off:(hf + 1) * 512],
                    lhsT=kT_t[ti][po:po + 64, kt * 128:(kt + 1) * 128],
                    rhs=qT_t[ti][po:po + 64, qg * 512 + off:(qg + 1) * 512],
                    start=True, stop=True,
                )
            ex = ex_pool.tile([128, 1024], BF16, tag="ex", name="ex")
            if off:
                ps_in = ps.rearrange("p (h q) -> p h q", q=512)[:, :, off:512]
                ex_out = ex.rearrange("p (h q) -> p h q", q=512)[:, :, off:512]
            else:
                ps_in, ex_out = ps, ex
            nc.scalar.activation(out=ex_out, in_=ps_in,
                                 func=mybir.ActivationFunctionType.Exp,
                                 scale=0.125)
            if diag:  # 0/1 mask on the 128-wide triangular sub-block only
                # two 2D muls (2x DVE mode; also lets avA start after just
                # the A-half mask)
                for hf in range(2):
                    exm = ex[:, hf * 512 + off:hf * 512 + off + 128]
                    nc.vector.tensor_mul(exm, exm,
                                         m01_t[:, hf * 128:(hf + 1) * 128])
            pending.append((kt, ex, off))
            if len(pending) > 2:  # lag 2: AV never waits on a fresh exp
                emit_av(*pending.pop(0))
            for u in inject.get(kt, []):
                u()

        def flush_tail():
            for item in pending:
                emit_av(*item)
            _stash(hp, qg, ti, avA, avB)
        return flush_tail

    def _stash(hp, qg, ti, avA, avB):
        hA, hB = 2 * hp, 2 * hp + 1
        if qg == LAST_QG and hp == HL // 2 - 1:
            # final slot: normalize inline via reciprocal + PE broadcast
            stg2 = rec_pool.tile([64, 512], F32, tag="stg2", name="stg2")
            nc.vector.memset(stg2, 1.0)
            for av, po, row in ((avA, 0, 0), (avB, 64, 32)):
                nc.vector.tensor_copy(
                    outT_t[ti][po:po + 64, qg * 512:(qg + 1) * 512],
                    av[0:64, :])
                nc.vector.tensor_copy(stg2[row:row + 1, :], av[64:65, :])
            rec2 = rec_pool.tile([64, 512], F32, tag="rec2", name="rec2")
            nc.vector.reciprocal_approx_fast(out=rec2, in_=stg2)
            recb2 = rec_pool.tile([64, 512], BF16, tag="recb2", name="recb2")
            nc.vector.tensor_copy(recb2, rec2)
            bc = av_pool.tile([128, 512], F32, tag="av", name="bc")
            nc.tensor.matmul(bc, lhsT=ones2, rhs=recb2, start=True, stop=True)
            for po in (0, 64):
                sl = outT_t[ti][po:po + 64, qg * 512:(qg + 1) * 512]
                nc.vector.tensor_mul(sl, sl, bc[po:po + 64, :])
        else:
            for av, h, po in ((avA, hA, 0), (avB, hB, 64)):
                nc.vector.tensor_copy(
                    outT_t[ti][po:po + 64, qg * 512:(qg + 1) * 512],
                    av[0:64, :])
                stg = rec_pool.tile([1, 512], F32, tag="stg", name="stg",
                                    bufs=4)
                nc.vector.tensor_copy(stg, av[64:65, :])
                nc.sync.dma_start(out=sums_dram[qg, h], in_=stg)

    # ---- batched normalization (DRAM-bounce broadcast) -------------------
    def _norm_heads(qg, heads):
        h0, nh = heads[0], len(heads)
        sums = rec_pool.tile([nh, 512], F32, tag=f"sums{nh}", name="sums")
        nc.sync.dma_start(out=sums, in_=sums_dram[qg, h0:h0 + nh])
        rec = rec_pool.tile([nh, 512], F32, tag=f"rec{nh}", name="rec")
        nc.vector.reciprocal_approx_fast(out=rec, in_=sums)
        recb = rec_pool.tile([nh, 512], BF16, tag=f"recb{nh}", name="recb")
        nc.vector.tensor_copy(recb, rec)
        nc.sync.dma_start(out=rec_dram[qg, h0:h0 + nh], in_=recb)
        for h in heads:
            ti, po = h // 2, 64 * (h % 2)
            bcs = rec_pool.tile([128, 512], BF16, tag="bcs", name="bcs")
            nc.sync.dma_start(
                out=bcs[po:po + 64, :],
                in_=rec_dram[qg, h:h + 1, :].to_broadcast([64, 512]))
            sl = outT_t[ti][po:po + 64, qg * 512:(qg + 1) * 512]
            nc.vector.tensor_mul(sl, sl, bcs[po:po + 64, :])

    def normalize(qg):
        _norm_heads(qg, list(range(HL)))

    def normalize_pair(qg, hp):
        _norm_heads(qg, [2 * hp, 2 * hp + 1])

    # ---- o-projection: y[s,:] partial ------------------------------------
    def oproj_h(st, hf):
        ps = fl_pool.tile([128, 512], F32, tag="fl", name="fps")
        for dt_ in range(NET):
            nc.tensor.matmul(
                ps,
                lhsT=outT_t[dt_][:, st * 128:(st + 1) * 128],
                rhs=woT_t[dt_][:, hf * 512:(hf + 1) * 512],
                start=(dt_ == 0), stop=(dt_ == NET - 1),
            )
        ysb = y_pool.tile([128, 512], BF16, tag="ysb", name="ysb", bufs=4)
        nc.vector.tensor_copy(ysb, ps)
        q = nc.sync if hf == 0 else nc.gpsimd
        q.dma_start(
            out=y[st * 128:(st + 1) * 128, hf * 512:(hf + 1) * 512],
            in_=ysb)

    def oproj(st):
        oproj_h(st, 0)
        oproj_h(st, 1)

    # ---- program order ----------------------------------------------------
    def qkQ(et, scg):
        return lambda: proj_qk(wqT_t, qT_t, et, scg)

    def qkK(et, scg):
        return lambda: proj_qk(wkT_t, kT_t, et, scg)

    def V(stp):
        return lambda: proj_v(stp)

    def O(st):
        return lambda: oproj(st)

    def N_(qg):
        return lambda: normalize(qg)

    proj_v(0)
    proj_v(1)
    proj_qk(wqT_t, qT_t, 0, 0)
    proj_qk(wkT_t, kT_t, 0, 0)

    # filler units injected inside each slot's kt loop (slot = (qg, hp) in
    # QG_ORDER-major, hp-minor order, s = visit index 0..15).  Deadlines:
    # qk(et,scg) before the first slot of a qg using scg that reads et;
    # v(stp) before any slot whose kt loop reaches st=2*stp; o(st) after
    # normalize of st's qg; normalize(qg) after all four qg stashes.
    fillers = {
        (0, 0): [qkQ(1, 0), qkK(1, 0)],
        (0, 1): [qkQ(2, 0), qkK(2, 0)],
        (0, 2): [qkQ(3, 0), qkK(3, 0)],
        (0, 3): [V(2), V(3)],
        (1, 0): [V(4), qkQ(0, 1), N_(0)],
        (1, 1): [V(5), qkK(0, 1)],
        (1, 2): [V(6), qkQ(1, 1)],
        (1, 3): [V(7), qkK(1, 1)],
        (3, 0): [qkQ(2, 1), qkK(2, 1), N_(1)],
        (3, 1): [qkQ(3, 1), qkK(3, 1)],
        (3, 2): [O(0), O(1), O(2)],
        (3, 3): [O(3), O(4), O(5)],
        (2, 0): [N_(3), O(6)],
        (2, 1): [O(7), O(12)],
        (2, 2): [O(13), O(14)],
        (2, 3): [O(15)],
    }
    post = {
        (2, 0): [lambda: normalize_pair(2, 0)],
        (2, 1): [lambda: normalize_pair(2, 1)],
        (2, 2): [lambda: normalize_pair(2, 2)],
    }
    for qg in QG_ORDER:
        for hp in range(HL // 2):
            flush_tail = attn(hp, qg, fillers.get((qg, hp), []))
            flush_tail()
            for f in post.get((qg, hp), []):
                f()
    for st in range(8, 12):  # qg2 (visited last) o-projections
        oproj(st)


def _build():
    nc = bacc.Bacc("TRN2", target_bir_lowering=False, debug=False,
                   num_devices=NCORES)
    xT = nc.dram_tensor("xT", [D, S], BF16, kind="ExternalInput").ap()
    wqT = nc.dram_tensor("wqT", [D, E], BF16, kind="ExternalInput").ap()
    wkT = nc.dram_tensor("wkT", [D, E], BF16, kind="ExternalInput").ap()
    wvT = nc.dram_tensor("wvT", [D, E], BF16, kind="ExternalInput").ap()
    woT = nc.dram_tensor("woT", [E, D], BF16, kind="ExternalInput").ap()
    m01 = nc.dram_tensor("m01", [128, 256], BF16, kind="ExternalInput").ap()
    y = nc.dram_tensor("y", [S, D], BF16, kind="ExternalOutput").ap()
    with tile.TileContext(nc) as tc:
        _mhsa_kernel(tc, y, xT, wqT, wkT, wvT, woT, m01)
    nc.compile()
    return nc


def get_compiled():
    global _compiled
    if _compiled is None:
        _compiled = _build()
    return _compiled


def _make_consts():
    # m01[k, qq] = 1 iff query qq >= key k within the 128-wide diagonal
    # sub-block; duplicated for the two packed heads.
    tri = np.triu(np.ones((128, 128), dtype=np.float32))
    m01 = np.concatenate([tri, tri], axis=1)
    return m01.astype(bf16)


def kernel(**inputs):
    global last_results
    x = np.asarray(inputs["in_features"], dtype=np.float32)
    w_q = np.asarray(inputs["w_q"], dtype=np.float32)
    w_k = np.asarray(inputs["w_k"], dtype=np.float32)
    w_v = np.asarray(inputs["w_v"], dtype=np.float32)
    w_o = np.asarray(inputs["w_o"], dtype=np.float32)

    nc = get_compiled()
    m01 = _make_consts()
    in_maps = []
    for c in range(NCORES):
        b, hg = divmod(c, 2)
        es = slice(hg * E, (hg + 1) * E)
        in_maps.append({
            "xT": x[b].T.astype(bf16),
            "wqT": w_q[es, :].T.astype(bf16),
            "wkT": w_k[es, :].T.astype(bf16),
            "wvT": w_v[es, :].T.astype(bf16),
            "woT": w_o[:, es].T.astype(bf16),
            "m01": m01,
        })
    res = run_bass_kernel_spmd(nc, in_maps, list(range(NCORES)))
    last_results = res
    y = np.zeros((B, S, D), dtype=np.float32)
    for c in range(NCORES):
        y[c // 2] += np.asarray(res.results[c]["y"], dtype=np.float32)
    return y
